# revision 14
# baseline (speedup 1.0000x reference)
"""BPR loss kernel for Trainium2 (Bass, raw engine streams), SPMD over 8 cores.

Reference computation (B=32, T=100, N=100000, S=1):
    pos  = output[b, t, labels[b, t]]
    neg  = output[b, t, neg_ids[b, t, 0]]
    per_t = log_sigmoid(pos - neg)                # = -softplus(neg - pos)
    per_user = sum_t(per_t * (t < x_len[b])) / x_len[b]
    loss = -mean_b(per_user)

Only 2 of the 100000 items per (b, t) are touched, so instead of streaming
the 1.28 GB logits tensor we gather exactly the needed scalars with indirect
(SWDGE) DMAs and do the tiny masked reduction on-chip.

HW indirect-DMA semantics (probed): each destination PARTITION consumes one
index (element units) from the offsets AP and receives dest_free_size
consecutive elements — so one instruction moves at most 128 scattered
scalars, and SWDGE costs ~1us FIXED per instruction (+0.34ns/descriptor).
Two structural moves minimize the instruction count:
  - masked terms (t >= x_len) contribute exactly 0, so only valid (user, t)
    pairs are gathered; users are LPT-balanced across the 8 cores by x_len
    so every core carries ~sum(x_len)/8 pairs.
  - valid pairs are packed densely into [128, J] slots (pair k -> partition
    k%128, column k//128, J = ceil(max_core_pairs/128)); pos values land in
    columns [0,J), matching negs in [J,2J) -> 2J gather instructions total
    (J=2 for typical inputs, vs 8 for the per-user-column layout).
Per-slot weights W[p, 4j+v] = (pair (p,j) belongs to core-user v) / x_len
are precomputed on host in fp16 and ride the same input DMA, so the masked
per-user reduction is a single-pass fp16 matmul acc[4j+v, j'] (host sums the
j==j' entries); accumulation stays fp32 in PSUM.

Perf structure: ONE packed input DMA (gather offsets + weights + ACT bias
columns; feeding biases from the packed input lets Bass's const memsets be
stripped); softplus(z) = Ln(Exp(z) + 1) with both ACT funcs sharing one
table (natural_log_exp_and_others — enforced by narrowing the table-picker's
view during build) and the scalar block starting directly at the table load
so it overlaps the gathers; DVE stream is just [sub, psum-copy];
Block(no_gpsimd_drain=True) + stripping the block-end EVSEM barrier leans on
the NEFF epilogue's own all-engine butterfly for the exit sync (gpsimd holds
its stream open on the output-DMA sem so the epilogue's semaphore clears
cannot clobber it early).
"""

import math
from contextlib import ExitStack

import numpy as np

B, T, N_ITEMS, S = 32, 100, 100000, 1
N_CORES = 8
BP = B // N_CORES      # users per core = 4
P = 128                # slot partitions

_CACHE = {}


def _build_nc(J):
    from concourse import bass, bass_isa, bacc, mybir

    f32 = mybir.dt.float32
    f16 = mybir.dt.float16
    i32 = mybir.dt.int32

    K = 2 * J                  # gather instructions / vals columns
    # packed words per row: gx(2J) w_f32(4J) one(1) zero(1)
    PKW = 6 * J + 2

    nc = bacc.Bacc()
    xs = nc.declare_dram_parameter("xs", [BP * T, N_ITEMS], f32, isOutput=False)
    pk = nc.declare_dram_parameter("pk", [P, PKW], i32, isOutput=False)
    res = nc.declare_dram_parameter("res", [1, 4 * J], f32, isOutput=True)

    with ExitStack() as stk:
        pk_t = stk.enter_context(nc.sbuf_tensor([P, PKW], i32))
        vals = stk.enter_context(nc.sbuf_tensor([P, K], f32))
        z = stk.enter_context(nc.sbuf_tensor([P, J], f32))
        ez = stk.enter_context(nc.sbuf_tensor([P, J], f32))
        sp = stk.enter_context(nc.sbuf_tensor([P, J], f32))
        prod = stk.enter_context(nc.sbuf_tensor([P, 4 * J], f32))
        red_sb = stk.enter_context(nc.sbuf_tensor([P, 4 * J], f32))

        gx_ap = pk_t[:, 0:K]
        w_ap = pk_t[:, K : K + 4 * J].bitcast(f32)      # [P, 4J], m = v*J+j
        one_ap = pk_t[:, PKW - 2 : PKW - 1].bitcast(f32)
        zero_ap = pk_t[:, PKW - 1 : PKW].bitcast(f32)

        with (
            nc.Block(no_gpsimd_drain=True) as block,
            nc.semaphore("s_dma") as s_dma,
            nc.semaphore("s_dge") as s_dge,
            nc.semaphore("s_v") as s_v,
            nc.semaphore("s_a") as s_a,
            nc.semaphore("s_g") as s_g,
        ):

            @block.sync
            def _(sync):
                sync.dma_start(out=pk_t[:, :], in_=pk[:, :]).then_inc(s_dma, 16)
                sync.wait_ge(s_g, 1)
                sync.dma_start(
                    out=res[:, :], in_=red_sb[0:1, :]
                ).then_inc(s_dma, 16)
                sync.wait_ge(s_dma, 32)

            @block.gpsimd
            def _(gpsimd):
                # 2J SWDGE gathers, one [128, 1] column each (the HW consumes
                # one host-precomputed absolute element index per partition).
                gpsimd.wait_ge(s_dma, 16)
                for c in range(K):
                    gpsimd.indirect_dma_start(
                        out=vals[:, c : c + 1],
                        out_offset=None,
                        in_=xs[:, :],
                        in_offset=bass.IndirectOffsetOnAxis(
                            ap=gx_ap[:, c : c + 1], axis=1
                        ),
                    ).then_inc(s_dge, 16)
                # cross-partition reduce replaces the PE matmul: keeping the
                # TensorEngine out of the kernel lets the NEFF epilogue's
                # (slow) semaphore-clear chain on it run DURING the kernel.
                gpsimd.wait_ge(s_v, 2)
                gpsimd.partition_all_reduce(
                    out_ap=red_sb[:, :], in_ap=prod[:, :],
                    channels=P, reduce_op=bass_isa.ReduceOp.add,
                ).then_inc(s_g, 1)
                # keep this stream parked until the output DMA lands: the NEFF
                # epilogue's semaphore clears on this engine include s_dma.
                gpsimd.wait_ge(s_dma, 32)

            @block.vector
            def _(vector):
                # z = neg - pos
                vector.wait_ge(s_dge, 16 * K)
                vector.tensor_sub(
                    out=z[:, :], in0=vals[:, J:K], in1=vals[:, 0:J]
                ).then_inc(s_v, 1)                                        # 1
                # prod[p, v*J+j] = W[p, v*J+j] * sp[p, j]
                vector.wait_ge(s_a, 2)
                vector.tensor_mul(
                    out=prod[:, :], in0=w_ap,
                    in1=sp[:, :].unsqueeze(1).broadcast_to((P, 4, J)),
                ).then_inc(s_v, 1)                                        # 2

            @block.scalar
            def _(scalar):
                # softplus(z) = Ln(Exp(z) + 1); Exp and Ln share one ACT
                # table whose load is placed at block entry (before Exp's
                # wait), overlapping the input DMA + gathers.
                scalar.wait_ge(s_v, 1)
                scalar.activation(
                    ez[:, :], z[:, :], mybir.ActivationFunctionType.Exp,
                    bias=zero_ap,
                ).then_inc(s_a, 1)
                scalar.wait_ge(s_a, 1)
                scalar.activation(
                    sp[:, :], ez[:, :], mybir.ActivationFunctionType.Ln,
                    bias=one_ap,
                ).then_inc(s_a, 1)

    _strip_const_memsets(nc)
    _strip_block_end_barrier(nc)
    _finalize_with_shared_act_table(nc)
    return nc


def _strip_const_memsets(nc):
    """Drop the unconditional Bass const-AP memsets (unused here: ACT biases
    come from the packed input). They would otherwise be the first 'useful'
    instructions the profiler counts, ~1.3us before the input DMA."""
    for f in nc.m.functions:
        for bb in f.blocks:
            insts = bb.instructions
            keep = [
                i
                for i in insts
                if not (
                    type(i).__name__ == "InstMemset"
                    and str(getattr(i.outs[0], "memref", "")).startswith("const-")
                )
            ]
            if len(keep) != len(insts):
                bb.instructions = keep


def _strip_block_end_barrier(nc):
    """Drop the sem-only all-engine barrier at block end (keep the drains).
    The NEFF epilogue runs its own all-engine butterfly immediately after,
    and every cross-engine dependency is already sem-ordered; gpsimd
    additionally holds its stream open on the output-DMA sem."""
    for f in nc.m.functions:
        for bb in f.blocks:
            if bb.name.endswith("_end"):
                bb.instructions = [
                    i
                    for i in bb.instructions
                    if type(i).__name__ != "InstEventSemaphore"
                ]


def _finalize_with_shared_act_table(nc):
    """Finalize with the ACT table-picker constrained so Exp and Ln both
    resolve to natural_log_exp_and_others (one load, no mid-kernel table
    swap). Table ids/order are untouched, so InstLoadActFuncSet ids still
    match the compiler's act_info.json. Patch is restored afterwards."""
    from concourse import bacc, hw_specs, mybir

    target = "natural_log_exp_and_others"
    orig = hw_specs.get_activation_tables

    def narrowed(arch):
        tabs = orig(arch)
        if target in tabs:
            for name, fns in tabs.items():
                if name != target:
                    fns.discard(mybir.ActivationFunctionType.Exp)
                    fns.discard(mybir.ActivationFunctionType.Ln)
        return tabs

    hw_specs.get_activation_tables = narrowed
    bacc.get_activation_tables = narrowed
    try:
        if not nc.is_finalized():
            nc.finalize()
    finally:
        hw_specs.get_activation_tables = orig
        bacc.get_activation_tables = orig


def _get_nc(J):
    if J not in _CACHE:
        _CACHE[J] = _build_nc(J)
    return _CACHE[J]


def _assign_users(x_lens):
    """LPT-balance the 32 users into 8 bins of 4 by x_len sum."""
    xl = np.asarray(x_lens).astype(np.int64)
    order = np.argsort(-xl, kind="stable")
    bins = [[] for _ in range(N_CORES)]
    sums = [0] * N_CORES
    for u in order:
        c = min(
            (c for c in range(N_CORES) if len(bins[c]) < BP),
            key=lambda c: sums[c],
        )
        bins[c].append(int(u))
        sums[c] += int(xl[u])
    return bins, max(sums)


def _make_in_maps(output, labels, x_lens, neg_ids, bins, J):
    output = np.asarray(output, dtype=np.float32)
    labels = np.asarray(labels).astype(np.int64)
    neg = np.asarray(neg_ids).astype(np.int64).reshape(B, T * S)
    xl = np.asarray(x_lens).astype(np.int64)

    K = 2 * J
    in_maps = []
    for users in bins:
        gx = np.zeros((P, K), np.int32)
        w = np.zeros((P, 4 * J), np.float32)
        k = 0
        for v, gu in enumerate(users):
            n = int(xl[gu])
            t = np.arange(n, dtype=np.int64)
            rowbase = (v * T + t) * N_ITEMS
            pos_idx = rowbase + labels[gu, :n]
            neg_idx = rowbase + neg[gu, :n]
            sl = np.arange(k, k + n)
            p, j = sl % P, sl // P
            gx[p, j] = pos_idx.astype(np.int32)
            gx[p, J + j] = neg_idx.astype(np.int32)
            w[p, v * J + j] = np.float32(1.0 / n)
            k += n
        pk = np.concatenate(
            [
                gx,
                w.view(np.int32),
                np.ones((P, 1), np.float32).view(np.int32),
                np.zeros((P, 1), np.int32),
            ],
            axis=1,
        )
        in_maps.append(
            {
                "xs": np.ascontiguousarray(output[users]).reshape(
                    BP * T, N_ITEMS
                ),
                "pk": np.ascontiguousarray(pk),
            }
        )
    return in_maps


def run(output, labels, x_lens, neg_ids, uids=None, trace=False):
    """Run the SPMD bass kernel; returns (loss_scalar, BassKernelResults)."""
    from concourse.bass_utils import run_bass_kernel_spmd

    bins, max_pairs = _assign_users(x_lens)
    J = max(1, math.ceil(max_pairs / P))
    nc = _get_nc(J)
    in_maps = _make_in_maps(output, labels, x_lens, neg_ids, bins, J)
    out = run_bass_kernel_spmd(nc, in_maps, list(range(N_CORES)), trace=trace)
    # res[0, v*J+j] accumulates user v's (positive) partial from column j.
    partials = []
    for c in range(N_CORES):
        r = out.results[c]["res"]
        for v in range(BP):
            partials.append(sum(r[0, v * J + j] for j in range(J)))
    loss = np.asarray(partials, dtype=np.float32).mean(dtype=np.float32)
    return np.float32(loss), out


def kernel(output, labels, x_lens, neg_ids, uids=None, **_ignored):
    loss, _ = run(output, labels, x_lens, neg_ids)
    return loss


# revision 19
# speedup vs baseline: 1.2689x; 1.2689x over previous
"""BPR loss kernel for Trainium2 (Bass, raw engine streams), SPMD over 8 cores.

Reference computation (B=32, T=100, N=100000, S=1):
    pos  = output[b, t, labels[b, t]]
    neg  = output[b, t, neg_ids[b, t, 0]]
    per_t = log_sigmoid(pos - neg)                # = -softplus(neg - pos)
    per_user = sum_t(per_t * (t < x_len[b])) / x_len[b]
    loss = -mean_b(per_user)

Only 2 of the 100000 items per (b, t) are touched, so instead of streaming
the 1.28 GB logits tensor we gather exactly the needed scalars with indirect
(SWDGE) DMAs and do the tiny masked reduction on-chip.

HW indirect-DMA semantics (probed): each destination PARTITION consumes one
index (element units) from the offsets AP and receives dest_free_size
consecutive elements — so one instruction moves at most 128 scattered
scalars, and SWDGE costs ~1us FIXED per instruction (+0.34ns/descriptor).
Two structural moves minimize the instruction count:
  - masked terms (t >= x_len) contribute exactly 0, so only valid (user, t)
    pairs are gathered; users are LPT-balanced across the 8 cores by x_len
    so every core carries ~sum(x_len)/8 pairs.
  - valid pairs are packed densely into [128, J] slots (pair k -> partition
    k%128, column k//128, J = ceil(max_core_pairs/128)); pos/neg values land
    in interleaved columns 2j/2j+1 -> 2J gather instructions total (J=2 for
    typical inputs, vs 8 for the per-user-column layout), and column pair
    j's softplus chain starts as soon as its two gathers land, hiding under
    pair j+1's descriptor generation.
Per-slot weights W[p, 4j+v] = (pair (p,j) belongs to core-user v) / x_len
are precomputed on host in fp16 and ride the same input DMA, so the masked
per-user reduction is a single-pass fp16 matmul acc[4j+v, j'] (host sums the
j==j' entries); accumulation stays fp32 in PSUM.

Perf structure: ONE packed input DMA (gather offsets + weights + ACT bias
columns; feeding biases from the packed input lets Bass's const memsets be
stripped); softplus(z) = Ln(Exp(z) + 1) with both ACT funcs sharing one
table (natural_log_exp_and_others — enforced by narrowing the table-picker's
view during build) and the scalar block starting directly at the table load
so it overlaps the gathers; DVE stream is just [sub, psum-copy];
Block(no_gpsimd_drain=True) + stripping the block-end EVSEM barrier leans on
the NEFF epilogue's own all-engine butterfly for the exit sync (gpsimd holds
its stream open on the output-DMA sem so the epilogue's semaphore clears
cannot clobber it early).
"""

import math
from contextlib import ExitStack

import numpy as np

B, T, N_ITEMS, S = 32, 100, 100000, 1
N_CORES = 8
BP = B // N_CORES      # users per core = 4
P = 128                # slot partitions

_CACHE = {}


def _build_nc(J):
    from concourse import bass, bacc, mybir

    f32 = mybir.dt.float32
    f16 = mybir.dt.float16
    i32 = mybir.dt.int32

    K = 2 * J                  # gather instructions / vals columns
    # packed words per row: gx(2J) w_f16(4J halves = 2J words) one(1) zero(1)
    PKW = 4 * J + 2

    nc = bacc.Bacc()
    xs = nc.declare_dram_parameter("xs", [BP * T, N_ITEMS], f32, isOutput=False)
    pk = nc.declare_dram_parameter("pk", [P, PKW], i32, isOutput=False)
    res = nc.declare_dram_parameter("res", [4 * J, J], f32, isOutput=True)

    with ExitStack() as stk:
        pk_t = stk.enter_context(nc.sbuf_tensor([P, PKW], i32))
        vals = stk.enter_context(nc.sbuf_tensor([P, K], f32))
        z = stk.enter_context(nc.sbuf_tensor([P, J], f32))
        ez = stk.enter_context(nc.sbuf_tensor([P, J], f32))
        sp = stk.enter_context(nc.sbuf_tensor([P, J], f16))
        res_sb = stk.enter_context(nc.sbuf_tensor([4 * J, J], f32))
        acc = stk.enter_context(nc.psum_tensor("acc", [4 * J, J], f32))

        gx_ap = pk_t[:, 0:K]
        w_ap = pk_t[:, K : 2 * K].bitcast(f16)          # [P, 4J]
        one_ap = pk_t[:, PKW - 2 : PKW - 1].bitcast(f32)
        zero_ap = pk_t[:, PKW - 1 : PKW].bitcast(f32)

        with (
            nc.Block(no_gpsimd_drain=True) as block,
            nc.semaphore("s_dma") as s_dma,
            nc.semaphore("s_dge") as s_dge,
            nc.semaphore("s_v") as s_v,
            nc.semaphore("s_a") as s_a,
            nc.semaphore("s_p") as s_p,
        ):

            @block.sync
            def _(sync):
                sync.dma_start(out=pk_t[:, :], in_=pk[:, :]).then_inc(s_dma, 16)
                sync.wait_ge(s_v, J + 1)
                sync.dma_start(out=res[:, :], in_=res_sb[:, :]).then_inc(s_dma, 16)
                sync.wait_ge(s_dma, 32)

            @block.gpsimd
            def _(gpsimd):
                # 2J SWDGE gathers, one [128, 1] column each (the HW consumes
                # one host-precomputed absolute element index per partition).
                # Column order is pair-interleaved (pos_j, neg_j) so the
                # downstream softplus pipeline can start after each pair.
                gpsimd.wait_ge(s_dma, 16)
                for c in range(K):
                    gpsimd.indirect_dma_start(
                        out=vals[:, c : c + 1],
                        out_offset=None,
                        in_=xs[:, :],
                        in_offset=bass.IndirectOffsetOnAxis(
                            ap=gx_ap[:, c : c + 1], axis=1
                        ),
                    ).then_inc(s_dge, 16)
                # keep this stream parked until the output DMA lands: the NEFF
                # epilogue's semaphore clears on this engine include s_dma.
                gpsimd.wait_ge(s_dma, 32)

            @block.vector
            def _(vector):
                # z_j = neg_j - pos_j as soon as pair j's gathers land; pair
                # 0's softplus chain hides under pair 1's gathers.
                for j in range(J):
                    vector.wait_ge(s_dge, 32 * (j + 1))
                    vector.tensor_sub(
                        out=z[:, j : j + 1],
                        in0=vals[:, 2 * j + 1 : 2 * j + 2],
                        in1=vals[:, 2 * j : 2 * j + 1],
                    ).then_inc(s_v, 1)                                    # j+1
                # PSUM -> SBUF
                vector.wait_ge(s_p, 1)
                vector.tensor_copy(out=res_sb[:, :], in_=acc[:, :]).then_inc(
                    s_v, 1
                )                                                         # J+1

            @block.scalar
            def _(scalar):
                # softplus(z) = Ln(Exp(z) + 1); Exp and Ln share one ACT
                # table whose load is placed at block entry (before Exp_0's
                # wait), overlapping the input DMA + gathers.
                for j in range(J):
                    scalar.wait_ge(s_v, j + 1)
                    scalar.activation(
                        ez[:, j : j + 1], z[:, j : j + 1],
                        mybir.ActivationFunctionType.Exp, bias=zero_ap,
                    ).then_inc(s_a, 1)
                    scalar.wait_ge(s_a, 2 * j + 1)
                    scalar.activation(
                        sp[:, j : j + 1], ez[:, j : j + 1],
                        mybir.ActivationFunctionType.Ln, bias=one_ap,
                    ).then_inc(s_a, 1)

            @block.tensor
            def _(tensor):
                # acc[m, n] = sum_p W[p, m] * sp[p, n]; host reads the
                # m = 4n+v entries. fp16 x fp16 -> single-pass matmul.
                tensor.wait_ge(s_dma, 16)
                tensor.wait_ge(s_a, 2 * J)
                tensor.matmul(
                    out=acc[:, :], lhsT=w_ap, rhs=sp[:, :],
                    start=True, stop=True,
                ).then_inc(s_p, 1)

    _strip_const_memsets(nc)
    _strip_block_end_barrier(nc)
    _finalize_with_shared_act_table(nc)
    return nc


def _strip_const_memsets(nc):
    """Drop the unconditional Bass const-AP memsets (unused here: ACT biases
    come from the packed input). They would otherwise be the first 'useful'
    instructions the profiler counts, ~1.3us before the input DMA."""
    for f in nc.m.functions:
        for bb in f.blocks:
            insts = bb.instructions
            keep = [
                i
                for i in insts
                if not (
                    type(i).__name__ == "InstMemset"
                    and str(getattr(i.outs[0], "memref", "")).startswith("const-")
                )
            ]
            if len(keep) != len(insts):
                bb.instructions = keep


def _strip_block_end_barrier(nc):
    """Drop the sem-only all-engine barrier at block end (keep the drains).
    The NEFF epilogue runs its own all-engine butterfly immediately after,
    and every cross-engine dependency is already sem-ordered; gpsimd
    additionally holds its stream open on the output-DMA sem."""
    for f in nc.m.functions:
        for bb in f.blocks:
            if bb.name.endswith("_end"):
                bb.instructions = [
                    i
                    for i in bb.instructions
                    if type(i).__name__ != "InstEventSemaphore"
                ]


def _finalize_with_shared_act_table(nc):
    """Finalize with the ACT table-picker constrained so Exp and Ln both
    resolve to natural_log_exp_and_others (one load, no mid-kernel table
    swap). Table ids/order are untouched, so InstLoadActFuncSet ids still
    match the compiler's act_info.json. Patch is restored afterwards."""
    from concourse import bacc, hw_specs, mybir

    target = "natural_log_exp_and_others"
    orig = hw_specs.get_activation_tables

    def narrowed(arch):
        tabs = orig(arch)
        if target in tabs:
            for name, fns in tabs.items():
                if name != target:
                    fns.discard(mybir.ActivationFunctionType.Exp)
                    fns.discard(mybir.ActivationFunctionType.Ln)
        return tabs

    hw_specs.get_activation_tables = narrowed
    bacc.get_activation_tables = narrowed
    try:
        if not nc.is_finalized():
            nc.finalize()
    finally:
        hw_specs.get_activation_tables = orig
        bacc.get_activation_tables = orig


def _get_nc(J):
    if J not in _CACHE:
        _CACHE[J] = _build_nc(J)
    return _CACHE[J]


def _assign_users(x_lens):
    """LPT-balance the 32 users into 8 bins of 4 by x_len sum."""
    xl = np.asarray(x_lens).astype(np.int64)
    order = np.argsort(-xl, kind="stable")
    bins = [[] for _ in range(N_CORES)]
    sums = [0] * N_CORES
    for u in order:
        c = min(
            (c for c in range(N_CORES) if len(bins[c]) < BP),
            key=lambda c: sums[c],
        )
        bins[c].append(int(u))
        sums[c] += int(xl[u])
    return bins, max(sums)


def _make_in_maps(output, labels, x_lens, neg_ids, bins, J):
    output = np.asarray(output, dtype=np.float32)
    labels = np.asarray(labels).astype(np.int64)
    neg = np.asarray(neg_ids).astype(np.int64).reshape(B, T * S)
    xl = np.asarray(x_lens).astype(np.int64)

    K = 2 * J
    in_maps = []
    for users in bins:
        gx = np.zeros((P, K), np.int32)
        w = np.zeros((P, 4 * J), np.float16)
        k = 0
        for v, gu in enumerate(users):
            n = int(xl[gu])
            t = np.arange(n, dtype=np.int64)
            rowbase = (v * T + t) * N_ITEMS
            pos_idx = rowbase + labels[gu, :n]
            neg_idx = rowbase + neg[gu, :n]
            sl = np.arange(k, k + n)
            p, j = sl % P, sl // P
            gx[p, 2 * j] = pos_idx.astype(np.int32)
            gx[p, 2 * j + 1] = neg_idx.astype(np.int32)
            w[p, 4 * j + v] = np.float16(1.0 / n)
            k += n
        pk = np.concatenate(
            [
                gx,
                w.view(np.int32),
                np.ones((P, 1), np.float32).view(np.int32),
                np.zeros((P, 1), np.int32),
            ],
            axis=1,
        )
        in_maps.append(
            {
                "xs": np.ascontiguousarray(output[users]).reshape(
                    BP * T, N_ITEMS
                ),
                "pk": np.ascontiguousarray(pk),
            }
        )
    return in_maps


def run(output, labels, x_lens, neg_ids, uids=None, trace=False):
    """Run the SPMD bass kernel; returns (loss_scalar, BassKernelResults)."""
    from concourse.bass_utils import run_bass_kernel_spmd

    bins, max_pairs = _assign_users(x_lens)
    J = max(1, math.ceil(max_pairs / P))
    nc = _get_nc(J)
    in_maps = _make_in_maps(output, labels, x_lens, neg_ids, bins, J)
    out = run_bass_kernel_spmd(nc, in_maps, list(range(N_CORES)), trace=trace)
    # res[4j+v, j] accumulates user v's (positive) partial from column j.
    partials = []
    for c in range(N_CORES):
        r = out.results[c]["res"]
        for v in range(BP):
            partials.append(sum(r[4 * j + v, j] for j in range(J)))
    loss = np.asarray(partials, dtype=np.float32).mean(dtype=np.float32)
    return np.float32(loss), out


def kernel(output, labels, x_lens, neg_ids, uids=None, **_ignored):
    loss, _ = run(output, labels, x_lens, neg_ids)
    return loss
